# revision 4
# baseline (speedup 1.0000x reference)
"""KGram embedding seq model kernel for 8 Trainium2 NeuronCores.

Computation (matching the reference):
    padded = concat(zeros(3, B), tokens)            # (S+3, B) token ids
    F[j]   = embed_table[padded_flat[j]]            # (2054, 341) gathered rows
    x[r]   = F_flat[(r + 2*(r&1))*341 : +1023]      # (2048, 1023) sliding windows
    h      = silu(x @ W1 + b1)                      # (2048, 1023)
    logits = h @ W2 + b2                            # (2048, 50257)

Sharding: data-parallel-free vocab split.  The embed gather and the first
matmul are small, so every core computes the full h; W2 is split
column-wise into 8 slices of 13 vocab tiles (512 cols each, zero-padded
past 50257) and each core produces logits for its slice.

Matmuls run in float32r (TF32-like PE mode: full fp32 memory format,
~1.5e-4 relative error, 4x the fp32 PE rate).  b1 is applied via the
ScalarE activation bias; b2 is folded into W2 as an extra contraction row
against a constant-ones row appended to h (K2 = 1024).
"""

import sys

sys.path.insert(0, "/opt/trn_rl_repo")

import numpy as np

import concourse.bass as bass
import concourse.mybir as mybir
import concourse.tile as tile
from concourse import bacc
from concourse import bass_utils

# Problem shapes
S, B = 1024, 2
K = 3
D = 341
HID = 1023           # K * D
K1 = 1024            # padded contraction for matmul 1 (zero row in W1)
K2 = 1024            # contraction for matmul 2 (HID + ones row for b2)
VOCAB = 50257
TOK = S * B          # 2048 output rows
NPAD = 2054          # S*B + K*B gathered embedding rows
N_CORES = 8
NT_PER_CORE = 13     # vocab tiles of 512 per core; 8*13*512 = 53248 >= 50257
NTILE = 512
VPAD = N_CORES * NT_PER_CORE * NTILE
TOKT = TOK // 128    # 16 token tiles
KT = 8               # contraction tiles of 128

_cached = {}


def _build():
    if "nc" in _cached:
        return _cached["nc"]

    f32 = mybir.dt.float32
    f32r = mybir.dt.float32r
    i32 = mybir.dt.int32

    nc = bacc.Bacc("TRN2", target_bir_lowering=False, debug=False,
                   num_devices=N_CORES)

    toks = nc.dram_tensor("toks", [NPAD, 1], i32, kind="ExternalInput")
    emb = nc.dram_tensor("emb", [VOCAB, D], f32r, kind="ExternalInput")
    w1 = nc.dram_tensor("w1", [K1, HID], f32r, kind="ExternalInput")
    b1 = nc.dram_tensor("b1", [HID, 1], f32, kind="ExternalInput")
    w2 = nc.dram_tensor("w2", [K2, NT_PER_CORE * NTILE], f32r,
                        kind="ExternalInput")
    ones = nc.dram_tensor("ones", [1, TOK], f32r, kind="ExternalInput")
    out = nc.dram_tensor("out", [TOK, NT_PER_CORE * NTILE], f32,
                         kind="ExternalOutput")

    with tile.TileContext(nc) as tc:
        with tc.tile_pool(name="dram", bufs=1, space="DRAM") as dram_pool, \
             tc.tile_pool(name="resident", bufs=1) as res_pool, \
             tc.tile_pool(name="gather", bufs=4) as gat_pool, \
             tc.tile_pool(name="psum1", bufs=4, space="PSUM") as psum1, \
             tc.tile_pool(name="psum2", bufs=4, space="PSUM") as psum2:

            # ---- stage 0: gather embedding rows for all padded tokens ----
            # F is a flat DRAM scratch of the 2054 gathered embedding rows.
            F = dram_pool.tile([NPAD * D], f32r)

            n_full = NPAD // 128            # 16 full gather tiles
            rem = NPAD - n_full * 128       # 6 leftover rows
            for i in range(n_full + 1):
                rows = 128 if i < n_full else rem
                idx = gat_pool.tile([128, 1], i32, tag="idx")
                nc.sync.dma_start(idx[:rows, :], toks.ap()[i * 128:i * 128 + rows, :])
                g = gat_pool.tile([128, D], f32r, tag="g")
                nc.gpsimd.indirect_dma_start(
                    out=g[:rows, :],
                    out_offset=None,
                    in_=emb.ap(),
                    in_offset=bass.IndirectOffsetOnAxis(ap=idx[:rows, :1], axis=0),
                )
                dst = bass.AP(F[:].tensor, i * 128 * D, [[D, rows], [1, D]])
                nc.sync.dma_start(dst, g[:rows, :])

            # ---- resident weights / h ----
            w1_sb = [res_pool.tile([128, HID], f32r, tag=f"w1_{k}", name=f"w1_{k}") for k in range(KT)]
            for k in range(KT):
                nc.sync.dma_start(w1_sb[k][:], w1.ap()[k * 128:(k + 1) * 128, :])
            b1_sb = [res_pool.tile([128, 1], f32, tag=f"b1_{m}", name=f"b1s_{m}") for m in range(KT)]
            for m in range(KT):
                rows = 128 if m < 7 else HID - 896
                nc.sync.dma_start(b1_sb[m][:rows, :], b1.ap()[m * 128:m * 128 + rows, :])

            # hT: 8 k-tiles of (128, 2048); tile 7 row 127 is the ones row
            # that multiplies the b2 row of w2.
            hT = [res_pool.tile([128, TOK], f32r, tag=f"hT_{k}", name=f"hT_{k}") for k in range(KT)]
            nc.sync.dma_start(hT[7][127:128, :], ones.ap())

            # ---- stage 1: h = silu(x @ W1 + b1), written as hT ----
            with tc.tile_pool(name="xt", bufs=16) as xt_pool:
                for n in range(4):                      # token slices of 512
                    xts = []
                    for k in range(KT):
                        xt = xt_pool.tile([128, NTILE], f32r, tag="xt")
                        base = 341 * NTILE * n + 128 * k
                        src_e = bass.AP(F[:].tensor, base, [[1, 128], [682, 256]])
                        src_o = bass.AP(F[:].tensor, base + 1023, [[1, 128], [682, 256]])
                        nc.sync.dma_start(xt[:, 0:NTILE:2], src_e)
                        nc.sync.dma_start(xt[:, 1:NTILE:2], src_o)
                        xts.append(xt)
                    for m in range(KT):                 # hid_out tiles
                        rows = 128 if m < 7 else HID - 896
                        ps = psum1.tile([128, NTILE], f32, tag="ps1")
                        for k in range(KT):
                            nc.tensor.matmul(ps[:rows, :],
                                             w1_sb[k][:, m * 128:m * 128 + rows],
                                             xts[k][:],
                                             start=(k == 0), stop=(k == KT - 1))
                        nc.scalar.activation(
                            hT[m][:rows, n * NTILE:(n + 1) * NTILE],
                            ps[:rows, :],
                            mybir.ActivationFunctionType.Silu,
                            bias=b1_sb[m][:rows, :],
                        )

            # ---- stage 2: logits = hT.T @ w2 ----
            with tc.tile_pool(name="w2", bufs=16) as w2_pool, \
                 tc.tile_pool(name="osb", bufs=8) as out_pool:
                for nt in range(NT_PER_CORE):
                    w2s = []
                    for k in range(KT):
                        w2t = w2_pool.tile([128, NTILE], f32r, tag="w2")
                        nc.sync.dma_start(
                            w2t[:],
                            w2.ap()[k * 128:(k + 1) * 128,
                                    nt * NTILE:(nt + 1) * NTILE])
                        w2s.append(w2t)
                    for m in range(TOKT):
                        ps = psum2.tile([128, NTILE], f32, tag="ps2")
                        for k in range(KT):
                            nc.tensor.matmul(ps[:],
                                             hT[k][:, m * 128:(m + 1) * 128],
                                             w2s[k][:],
                                             start=(k == 0), stop=(k == KT - 1))
                        ot = out_pool.tile([128, NTILE], f32, tag="osb")
                        nc.scalar.activation(ot[:], ps[:],
                                             mybir.ActivationFunctionType.Copy)
                        nc.sync.dma_start(
                            out.ap()[m * 128:(m + 1) * 128,
                                     nt * NTILE:(nt + 1) * NTILE],
                            ot[:])

    nc.finalize()
    _cached["nc"] = nc
    return nc


def kernel(**inputs) -> np.ndarray:
    tokens_seq = np.asarray(inputs["tokens_seq"])
    embed_table = np.asarray(inputs["embed_table"], dtype=np.float32)
    W1 = np.asarray(inputs["W1"], dtype=np.float32)
    b1 = np.asarray(inputs["b1"], dtype=np.float32)
    W2 = np.asarray(inputs["W2"], dtype=np.float32)
    b2 = np.asarray(inputs["b2"], dtype=np.float32)

    # host-side input prep (sharding + padding only)
    padded = np.concatenate(
        [np.zeros((K, B), dtype=np.int64), tokens_seq.astype(np.int64)], axis=0)
    toks = padded.reshape(-1, 1).astype(np.int32)              # (2054, 1)

    w1p = np.concatenate([W1, np.zeros((1, HID), np.float32)], axis=0)  # (1024, 1023)
    w2a = np.concatenate([W2, b2[None, :]], axis=0)            # (1024, 50257)
    w2p = np.zeros((K2, VPAD), np.float32)
    w2p[:, :VOCAB] = w2a

    nc = _build()
    width = NT_PER_CORE * NTILE
    in_maps = []
    for c in range(N_CORES):
        in_maps.append({
            "toks": toks,
            "emb": embed_table,
            "w1": w1p,
            "b1": b1.reshape(HID, 1),
            "w2": np.ascontiguousarray(w2p[:, c * width:(c + 1) * width]),
            "ones": np.ones((1, TOK), np.float32),
        })

    res = bass_utils.run_bass_kernel_spmd(nc, in_maps, core_ids=list(range(N_CORES)))

    logits = np.empty((TOK, VOCAB), np.float32)
    for c in range(N_CORES):
        lo = c * width
        hi = min((c + 1) * width, VOCAB)
        if lo >= VOCAB:
            continue
        logits[:, lo:hi] = res.results[c]["out"][:, :hi - lo]
    return logits.reshape(S, B, VOCAB)
